# revision 3
# baseline (speedup 1.0000x reference)
"""DyGraphTransformer forward on 8 trn2 NeuronCores (Bass/Tile).

Sequence-parallel: each core owns 64 of the 512 rows (queries / residual
stream).  K/V are AllGathered per layer.  The Graphormer attention bias is
computed once per core for its 64 query rows via one-hot matmul gathers of
two tiny projected tables (edge / edge-dist embeddings renormed and
projected to per-head biases on device), stored as exp(bias) and folded
into softmax multiplicatively (scores are small, so softmax safely skips
the max-subtraction; normalization happens after A@V via a ones column).

Heavy matmuls run in fp32 (exact); attention probabilities/V are bf16.
ACT engine stays on the exp_and_others table set the whole kernel (exp,
tanh-gelu, copies); LN rsqrt is computed on DVE via bit-trick + Newton.
"""

import sys

sys.path.insert(0, "/opt/trn_rl_repo")

import contextlib

import numpy as np

import concourse.bacc as bacc
import concourse.bass as bass
import concourse.tile as tile
from concourse import mybir
from concourse.bass_utils import run_bass_kernel_spmd

# model dims
N, F, H, NH, L, W = 512, 256, 256, 8, 6, 2
DK = H // NH                 # 32
EEN_W = 32                   # edge_encode table entries
EDN = 128                    # edge_dist table entries
NC = 8                       # cores
T = N // NC                  # 64 tokens per core
NJT = N // 128               # 4 j-tiles
LN_EPS = 1e-5
SCALE = DK ** -0.5

F32 = mybir.dt.float32
BF16 = mybir.dt.bfloat16
I32 = mybir.dt.int32
AL = mybir.AluOpType
AF = mybir.ActivationFunctionType

GC1 = 0.7978845608028654     # sqrt(2/pi)
GC2 = GC1 * 0.044715

_CACHE = {}


def _bcast_row(dram_ap, p):
    """1-D DRAM AP [Hf] -> broadcast AP [p, Hf]."""
    return bass.AP(tensor=dram_ap.tensor, offset=dram_ap.offset,
                   ap=[[0, p]] + [list(x) for x in dram_ap.ap])


def _gbcast(dram_t, rep, ncols):
    """DRAM [G, ncols] -> [G*rep partitions, ncols], each row replicated."""
    ap = dram_t.ap()
    g = ap.ap[0][1]
    return bass.AP(tensor=ap.tensor, offset=ap.offset,
                   ap=[[ap.ap[0][0], g], [0, rep], [1, ncols]])


def build(debug=False, repeats=1, no_cc=False, skip=()):
    nc = bacc.Bacc("TRN2", target_bir_lowering=False, debug=False,
                   num_devices=NC)

    # ---------------- DRAM I/O ----------------
    xT_in = nc.dram_tensor("xT", [2, 128, T], F32, kind="ExternalInput")
    wfeat_in = nc.dram_tensor("w_feat", [2, 128, H], F32, kind="ExternalInput")
    bfeat_in = nc.dram_tensor("b_feat", [H], F32, kind="ExternalInput")
    e_in = nc.dram_tensor("edge_emb", [EEN_W, H], F32, kind="ExternalInput")
    eT_in = nc.dram_tensor("edge_embT", [2, 128, EEN_W], F32, kind="ExternalInput")
    ed_in = nc.dram_tensor("edge_dist_emb", [EDN, H], F32, kind="ExternalInput")
    edT_in = nc.dram_tensor("edge_dist_embT", [2, 128, EDN], F32, kind="ExternalInput")
    wee_in = nc.dram_tensor("w_ee", [2, 128, NH], F32, kind="ExternalInput")
    wed_in = nc.dram_tensor("w_ed", [2, 128, NH], F32, kind="ExternalInput")
    bee_in = nc.dram_tensor("bee32", [32], F32, kind="ExternalInput")
    bed_in = nc.dram_tensor("bed32", [32], F32, kind="ExternalInput")
    w_names = ["Wq", "Wk", "Wv", "Wo", "W1", "W2"]
    w_ins = {n: nc.dram_tensor(n, [L, 2, 128, H], F32, kind="ExternalInput")
             for n in w_names}
    b_names = ["bq", "bk", "bv", "bo", "b1", "b2", "ln1_s", "ln1_b",
               "ln2_s", "ln2_b"]
    b_ins = {n: nc.dram_tensor(n, [L, H], F32, kind="ExternalInput")
             for n in b_names}
    bmisc_in = nc.dram_tensor("bmisc", [L, 4, H], F32, kind="ExternalInput")
    t1a_in = nc.dram_tensor("t1a", [8, 4096], BF16, kind="ExternalInput")
    t1b_in = nc.dram_tensor("t1b", [8, 4096], BF16, kind="ExternalInput")
    t2_in = nc.dram_tensor("t2", [4, 8192], BF16, kind="ExternalInput")
    ident_in = nc.dram_tensor("ident", [128, 128], F32, kind="ExternalInput")
    iota16_in = nc.dram_tensor("iota16", [128], F32, kind="ExternalInput")
    iota32_in = nc.dram_tensor("iota32", [128], F32, kind="ExternalInput")

    out_t = nc.dram_tensor("out", [T, H], F32, kind="ExternalOutput")
    dbg = {}
    if debug:
        for name, shape in [("h0", [T, H]), ("p1", [EEN_W, NH]),
                            ("p2", [EDN, NH]), ("eb", [128, NH * NJT * T]),
                            ("y1", [T, H]), ("kT", [128, 2 * N]),
                            ("sc0", [128, NJT * T]), ("pr0", [128, T]),
                            ("o0", [T, H]), ("h1", [T, H])]:
            dbg[name] = nc.dram_tensor("dbg_" + name, shape, F32,
                                       kind="ExternalOutput")

    KT_WORDS = H * T              # 16384 f32
    V_WORDS = T * NH * 33 // 2    # bf16 v pre-interleaved with ones cols
    CC_WORDS = KT_WORDS + V_WORDS
    cc_ins = [nc.dram_tensor(f"cc_in{i}", [CC_WORDS], F32)
              for i in range(repeats * L)]
    cc_outs = [nc.dram_tensor(f"cc_out{i}", [NC, CC_WORDS], F32,
                              addr_space="Shared") for i in range(repeats * L)]

    with tile.TileContext(nc) as tc:
        ctx = contextlib.ExitStack()
        with ctx:
            const = ctx.enter_context(tc.tile_pool(name="const", bufs=1))
            wpool = ctx.enter_context(tc.tile_pool(name="weights", bufs=1))
            small = ctx.enter_context(tc.tile_pool(name="small", bufs=2))
            psMM = ctx.enter_context(tc.tile_pool(name="psMM", bufs=2, space="PSUM"))
            psSC = ctx.enter_context(tc.tile_pool(name="psSC", bufs=2, space="PSUM"))
            psO = ctx.enter_context(tc.tile_pool(name="psO", bufs=2, space="PSUM"))

            # ---------------- constants ----------------
            ident = const.tile([128, 128], F32)
            nc.sync.dma_start(out=ident, in_=ident_in[:, :])
            iota16f = const.tile([128, 1], F32)
            nc.sync.dma_start(out=iota16f, in_=iota16_in.ap().rearrange("(p o) -> p o", o=1))
            iota32f = const.tile([128, 1], F32)
            nc.sync.dma_start(out=iota32f, in_=iota32_in.ap().rearrange("(p o) -> p o", o=1))
            magic = const.tile([128, 1], I32)
            nc.vector.memset(magic, 0x5F3759DF)
            bee32 = const.tile([128, 1], F32)
            nc.sync.dma_start(out=bee32[:32], in_=bee_in.ap().rearrange("(p o) -> p o", o=1))
            bed32 = const.tile([128, 1], F32)
            nc.sync.dma_start(out=bed32[:32], in_=bed_in.ap().rearrange("(p o) -> p o", o=1))
            bsum32 = const.tile([128, 1], F32)
            nc.vector.tensor_tensor(out=bsum32[:32], in0=bee32[:32],
                                    in1=bed32[:32], op=AL.add)

            wfeat = const.tile([128, 2, H], F32)
            nc.sync.dma_start(out=wfeat,
                              in_=wfeat_in.ap().rearrange("a p f -> p a f"))
            bfeat_r = const.tile([128, H], F32)
            nc.sync.dma_start(out=bfeat_r[:T], in_=_bcast_row(bfeat_in.ap(), T))
            xT = const.tile([128, 2, T], F32)
            nc.sync.dma_start(out=xT, in_=xT_in.ap().rearrange("a p t -> p a t"))

            wsb = {}
            for n in w_names:
                tl = wpool.tile([128, L, 2, H], F32, tag="w_" + n)
                nc.sync.dma_start(out=tl,
                                  in_=w_ins[n].ap().rearrange("l a p f -> p l a f"))
                wsb[n] = tl
            bsb = {}
            for n in b_names:
                tl = wpool.tile([128, L, 2], F32, tag="b_" + n)
                nc.sync.dma_start(
                    out=tl, in_=b_ins[n].ap().rearrange("l (a p) -> p l a", p=128))
                bsb[n] = tl
            bq_sc = wpool.tile([128, L, 2], F32, tag="b_bqsc")
            nc.vector.tensor_scalar(
                out=bq_sc.rearrange("p l a -> p (l a)"),
                in0=bsb["bq"].rearrange("p l a -> p (l a)"),
                scalar1=SCALE, scalar2=None, op0=AL.mult)

            # ---------------- helpers ----------------
            def rsqrt_col(u_ap, p, tagp, iters=2):
                """rsqrt of f32 column [p,1] via bit trick + Newton on DVE."""
                ki = small.tile([128, 1], I32, tag=tagp + "ki")
                nc.vector.tensor_scalar(out=ki[:p], in0=u_ap.bitcast(I32),
                                        scalar1=1, scalar2=None,
                                        op0=AL.logical_shift_right)
                z = small.tile([128, 1], F32, tag=tagp + "z")
                nc.vector.tensor_tensor(out=z[:p].bitcast(I32), in0=magic[:p],
                                        in1=ki[:p], op=AL.subtract)
                t = small.tile([128, 1], F32, tag=tagp + "t")
                for _ in range(iters):
                    nc.vector.tensor_scalar(out=t[:p], in0=z[:p], scalar1=z[:p],
                                            scalar2=u_ap, op0=AL.mult, op1=AL.mult)
                    nc.vector.tensor_scalar(out=t[:p], in0=t[:p], scalar1=-0.5,
                                            scalar2=1.5, op0=AL.mult, op1=AL.add)
                    nc.vector.tensor_tensor(out=z[:p], in0=z[:p], in1=t[:p],
                                            op=AL.mult)
                return z

            def layernorm_stats(h_ap, tagp):
                stats = small.tile([128, 6], F32, tag=tagp + "st")
                nc.vector.bn_stats(out=stats[:T], in_=h_ap)
                mv = small.tile([128, 2], F32, tag=tagp + "mv")
                nc.vector.bn_aggr(out=mv[:T], in_=stats[:T])
                u = small.tile([128, 1], F32, tag=tagp + "u")
                nc.vector.tensor_scalar(out=u[:T], in0=mv[:T, 1:2],
                                        scalar1=LN_EPS, scalar2=None, op0=AL.add)
                rstd = rsqrt_col(u[:T], T, tagp)
                return mv, rstd

            for rep in range(repeats):
                # =====================================================
                # Stage 1: P1 [32,8], P2 [128,8]  (renormed, projected tables)
                # =====================================================
                bctx = contextlib.ExitStack()
                bb = bctx.enter_context(tc.tile_pool(name=f"biasbuild{rep}", bufs=1))
                psOH = bctx.enter_context(tc.tile_pool(name=f"psOH{rep}", bufs=2, space="PSUM"))

                def build_table(nat_in, tT_in, wp_in, n_e, tagp):
                    emb = bb.tile([128, H], F32, tag=tagp + "nat")
                    nc.sync.dma_start(out=emb[:n_e], in_=nat_in[:, :])
                    embT = bb.tile([128, 2, n_e], F32, tag=tagp + "T")
                    nc.sync.dma_start(out=embT,
                                      in_=tT_in.ap().rearrange("a p e -> p a e"))
                    wp = bb.tile([128, 2, NH], F32, tag=tagp + "w")
                    nc.sync.dma_start(out=wp,
                                      in_=wp_in.ap().rearrange("a p h -> p a h"))
                    sq = bb.tile([128, H], F32, tag=tagp + "sq")
                    nc.vector.tensor_tensor(out=sq[:n_e], in0=emb[:n_e],
                                            in1=emb[:n_e], op=AL.mult)
                    s = bb.tile([128, 1], F32, tag=tagp + "s")
                    nc.vector.tensor_reduce(out=s[:n_e], in_=sq[:n_e],
                                            axis=mybir.AxisListType.X, op=AL.add)
                    rs = rsqrt_col(s[:n_e], n_e, tagp)
                    nrm = bb.tile([128, 1], F32, tag=tagp + "n")
                    nc.vector.tensor_scalar(out=nrm[:n_e], in0=s[:n_e],
                                            scalar1=rs[:n_e], scalar2=1e-7,
                                            op0=AL.mult, op1=AL.add)
                    nc.vector.tensor_scalar(out=nrm[:n_e], in0=nrm[:n_e],
                                            scalar1=1.0, scalar2=None, op0=AL.max)
                    f = bb.tile([128, 1], F32, tag=tagp + "f")
                    nc.vector.reciprocal(out=f[:n_e], in_=nrm[:n_e])
                    pT_ps = psMM.tile([NH, 512], F32, tag="mm")
                    for a in range(2):
                        nc.tensor.matmul(pT_ps[:, :n_e], wp[:, a], embT[:, a],
                                         start=(a == 0), stop=(a == 1))
                    pT_sb = bb.tile([NH, 512], F32, tag=tagp + "pTs")
                    nc.scalar.activation(pT_sb[:, :n_e], pT_ps[:, :n_e], AF.Copy)
                    p_ps = psMM.tile([128, NH], F32, tag="mm")
                    nc.tensor.transpose(p_ps[:n_e], pT_sb[:NH, :n_e],
                                        ident[:NH, :NH])
                    p_sb = bb.tile([128, NH], F32, tag=tagp + "ps")
                    nc.vector.tensor_scalar(out=p_sb[:n_e], in0=p_ps[:n_e],
                                            scalar1=f[:n_e], scalar2=None,
                                            op0=AL.mult)
                    p_bf = bb.tile([128, NH], BF16, tag=tagp + "pbf")
                    nc.vector.tensor_copy(out=p_bf[:n_e], in_=p_sb[:n_e])
                    return p_sb, p_bf

                p1f, p1 = build_table(e_in, eT_in, wee_in, EEN_W, "t1")
                p2f, p2 = build_table(ed_in, edT_in, wed_in, EDN, "t2")
                if debug and rep == 0:
                    nc.sync.dma_start(out=dbg["p1"][:, :], in_=p1f[:EEN_W])
                    nc.sync.dma_start(out=dbg["p2"][:, :], in_=p2f[:EDN])

                # =====================================================
                # Stage 2: block-diagonal lhsT tables (bf16)
                #   T1 pass q in {0,1}: lhsT [128,64]: [16g+e', 8h+g] = P1[16q+e', h]
                #   T2 pass q in {0..3}: lhsT [128,32]: [32g+e', 4h+g] = P2[32q+e', h]
                # =====================================================
                t1_lhs = []
                for q in range(2):
                    tl = const.tile([128, 8, 8], BF16, tag=f"t1lhs{q}")
                    nc.vector.memset(tl.rearrange("p a b -> p (a b)"), 0.0)
                    for g in range(8):
                        nc.sync.dma_start(out=tl[16 * g:16 * g + 16, :, g],
                                          in_=p1[16 * q:16 * q + 16])
                    t1_lhs.append(tl.rearrange("p a b -> p (a b)"))
                t2_lhs = []
                for q in range(4):
                    tl = const.tile([128, 8, 4], BF16, tag=f"t2lhs{q}")
                    nc.vector.memset(tl.rearrange("p a b -> p (a b)"), 0.0)
                    for g in range(4):
                        nc.sync.dma_start(out=tl[32 * g:32 * g + 32, :, g],
                                          in_=p2[32 * q:32 * q + 32])
                    t2_lhs.append(tl.rearrange("p a b -> p (a b)"))

                # =====================================================
                # Stage 3: one-hot gathers -> head-major exp tables (bf16)
                # =====================================================
                t1a_idx = bb.tile([128, 4096], BF16, tag="t1a_idx")
                t1b_idx = bb.tile([128, 4096], BF16, tag="t1b_idx")
                t2_idx = bb.tile([128, 8192], BF16, tag="t2_idx")
                if "idx" not in skip:
                    nc.sync.dma_start(out=t1a_idx, in_=_gbcast(t1a_in, 16, 4096))
                    nc.scalar.dma_start(out=t1b_idx, in_=_gbcast(t1b_in, 16, 4096))
                    nc.sync.dma_start(out=t2_idx, in_=_gbcast(t2_in, 32, 8192))
                else:
                    nc.vector.memset(t1a_idx, 1.0)
                    nc.vector.memset(t1b_idx, 1.0)
                    nc.vector.memset(t2_idx, 1.0)

                t1a_hm = bb.tile([64, 4096], BF16, tag="t1a_hm")
                t1b_hm = bb.tile([64, 4096], BF16, tag="t1b_hm")
                t2_hm = bb.tile([32, 8192], BF16, tag="t2_hm")

                def onehot_gather(idx_tile, lhs_list, n_sub, iota, ncols, out_hm,
                                  mrows, scale, bias_ap, tagp):
                    npass = len(lhs_list)
                    for ch in range(ncols // 512):
                        ps = psOH.tile([64, 512], F32, tag="oh")
                        for q in range(npass):
                            oh = bb.tile([128, 512], BF16, tag="ohc")
                            nc.vector.tensor_scalar(
                                out=oh, in0=idx_tile[:, 512 * ch:512 * (ch + 1)],
                                scalar1=float(n_sub * q), scalar2=iota,
                                op0=AL.subtract, op1=AL.is_equal)
                            nc.tensor.matmul(ps[:mrows], lhs_list[q], oh,
                                             start=(q == 0), stop=(q == npass - 1))
                        nc.scalar.activation(
                            out_hm[:, 512 * ch:512 * (ch + 1)], ps[:mrows], AF.Exp,
                            bias=bias_ap if bias_ap is not None else 0.0,
                            scale=scale)

                if "gather" not in skip:
                    onehot_gather(t1a_idx, t1_lhs, 16, iota16f, 4096, t1a_hm, 64,
                                  0.5, None, "t1a")
                    onehot_gather(t1b_idx, t1_lhs, 16, iota16f, 4096, t1b_hm, 64,
                                  0.5, None, "t1b")
                    onehot_gather(t2_idx, t2_lhs, 32, iota32f, 8192, t2_hm, 32,
                                  1.0, bsum32[:32], "t2")
                else:
                    nc.vector.memset(t1a_hm, 1.0)
                    nc.vector.memset(t1b_hm, 1.0)
                    nc.vector.memset(t2_hm, 1.0)

                # =====================================================
                # Stage 4: reorient to [j, i] per head; eb = t1a*t1b*t2
                # =====================================================
                eb = const.tile([128, NH, NJT, T], BF16, tag="eb")
                ebt = bb.tile([128, NH, NJT, T], BF16, tag="ebt")
                if "reorient" in skip:
                    nc.vector.memset(eb.rearrange("p h j t -> p (h j t)"), 1.0)
                    nc.vector.memset(ebt.rearrange("p h j t -> p (h j t)"), 1.0)
                for h in (range(NH) if "reorient" not in skip else []):
                    nc.sync.dma_start(
                        out=eb[:, h].rearrange("p j t -> p (j t)"),
                        in_=t2_hm[4 * h:4 * h + 4].rearrange(
                            "g (jj r) -> g jj r", jj=32, r=NJT * T))
                    nc.scalar.dma_start(
                        out=ebt[:, h].rearrange("p j t -> p (j t)"),
                        in_=t1a_hm[8 * h:8 * h + 8].rearrange(
                            "g (jj r) -> g jj r", jj=16, r=NJT * T))
                nc.vector.tensor_tensor(
                    out=eb.rearrange("p h j t -> p (h j t)"),
                    in0=eb.rearrange("p h j t -> p (h j t)"),
                    in1=ebt.rearrange("p h j t -> p (h j t)"), op=AL.mult)
                for h in (range(NH) if "reorient" not in skip else []):
                    nc.scalar.dma_start(
                        out=ebt[:, h].rearrange("p j t -> p (j t)"),
                        in_=t1b_hm[8 * h:8 * h + 8].rearrange(
                            "g (jj r) -> g jj r", jj=16, r=NJT * T))
                nc.vector.tensor_tensor(
                    out=eb.rearrange("p h j t -> p (h j t)"),
                    in0=eb.rearrange("p h j t -> p (h j t)"),
                    in1=ebt.rearrange("p h j t -> p (h j t)"), op=AL.mult)
                if debug and rep == 0:
                    ebf = bb.tile([128, NH * NJT * T], F32, tag="ebf")
                    nc.vector.tensor_copy(out=ebf,
                                          in_=eb.rearrange("p h j t -> p (h j t)"))
                    nc.sync.dma_start(out=dbg["eb"][:, :], in_=ebf)

                bctx.close()
                bctx2 = contextlib.ExitStack()
                work = bctx2.enter_context(tc.tile_pool(name=f"work{rep}", bufs=2))

                # =====================================================
                # Stage 5: h0 = x @ Wfeat + b  (token-major [64,256])
                # =====================================================
                h_sb = const.tile([128, H], F32, tag="resid")
                h_ps = psMM.tile([64, H], F32, tag="mm")
                for a in range(2):
                    nc.tensor.matmul(h_ps, xT[:, a], wfeat[:, a],
                                     start=(a == 0), stop=(a == 1))
                nc.vector.tensor_tensor(out=h_sb[:T], in0=h_ps, in1=bfeat_r[:T],
                                        op=AL.add)
                if debug and rep == 0:
                    nc.sync.dma_start(out=dbg["h0"][:, :], in_=h_sb[:T])

                # =====================================================
                # Stage 6: layers
                # =====================================================
                for l in range(L):
                    # ---- LN1 -> yT (hidden-major, ln affine folded in evict) ----
                    mv, rstd = layernorm_stats(h_sb[:T], f"ln1")
                    y = work.tile([128, H], F32, tag="y1")
                    nc.vector.tensor_scalar(out=y[:T], in0=h_sb[:T],
                                            scalar1=mv[:T, 0:1], scalar2=rstd[:T],
                                            op0=AL.subtract, op1=AL.mult)
                    yT = work.tile([128, 2, T], F32, tag="y1T")
                    for a in range(2):
                        tp = psMM.tile([128, T], F32, tag="mm")
                        nc.tensor.transpose(tp, y[:T, 128 * a:128 * (a + 1)],
                                            ident[:T, :T])
                        nc.scalar.activation(yT[:, a], tp, AF.Identity,
                                             bias=bsb["ln1_b"][:, l, a:a + 1],
                                             scale=bsb["ln1_s"][:, l, a:a + 1])
                    if debug and rep == 0 and l == 0:
                        yb = work.tile([128, H], F32, tag="ydbg")
                        for a in range(2):
                            tp2 = psMM.tile([64, 128], F32, tag="mm")
                            nc.tensor.transpose(tp2, yT[:, a], ident[:, :])
                            nc.vector.tensor_copy(
                                out=yb[:T, 128 * a:128 * (a + 1)], in_=tp2)
                        nc.sync.dma_start(out=dbg["y1"][:, :], in_=yb[:T])

                    # ---- q^T, k^T hidden-major ----
                    qT = work.tile([128, 2, T], F32, tag="qT")
                    kT = work.tile([128, 2, T], F32, tag="kT")
                    for (dst, wn, bt, sc) in [(qT, "Wq", bq_sc, SCALE),
                                              (kT, "Wk", None, 1.0)]:
                        for m in range(2):
                            pp = psMM.tile([128, T], F32, tag="mm")
                            for a in range(2):
                                nc.tensor.matmul(
                                    pp, wsb[wn][:, l, a, 128 * m:128 * (m + 1)],
                                    yT[:, a], start=(a == 0), stop=(a == 1))
                            bias_ap = (bt if bt is not None else bsb["bk"])[:, l, m:m + 1]
                            nc.scalar.activation(dst[:, m], pp, AF.Identity,
                                                 bias=bias_ap, scale=sc)
                    # ---- misc per-free-dim biases (bv, bo, b1, b2) ----
                    bm_r = work.tile([128, 4, H], F32, tag="bm_r")
                    nc.scalar.dma_start(
                        out=bm_r[:T],
                        in_=_bcast_row(bmisc_in.ap()[l].rearrange("a f -> a f"), T))
                    # ---- v token-major bf16, pre-interleaved [h, 33] + ones ----
                    v_ps = psMM.tile([64, H], F32, tag="mm")
                    for a in range(2):
                        nc.tensor.matmul(v_ps, yT[:, a], wsb["Wv"][:, l, a],
                                         start=(a == 0), stop=(a == 1))
                    v_sb = work.tile([64, NH, 33], BF16, tag="v_sb")
                    nc.vector.memset(v_sb.rearrange("p a b -> p (a b)"), 1.0)
                    for h in range(NH):
                        nc.vector.tensor_tensor(
                            out=v_sb[:, h, 0:32], in0=v_ps[:, 32 * h:32 * (h + 1)],
                            in1=bm_r[:T, 0, 32 * h:32 * (h + 1)], op=AL.add)

                    # ---- allgather k^T (f32) + v (bf16) ----
                    nc.sync.dma_start(
                        out=cc_ins[rep * L + l].ap()[0:KT_WORDS].rearrange(
                            "(p f) -> p f", p=128),
                        in_=kT.rearrange("p a t -> p (a t)"))
                    nc.scalar.dma_start(
                        out=cc_ins[rep * L + l].ap()[KT_WORDS:CC_WORDS].rearrange(
                            "(p f) -> p f", p=T).bitcast(BF16),
                        in_=v_sb.rearrange("p a b -> p (a b)"))
                    nc.gpsimd.collective_compute(
                        "AllGather", AL.bypass,
                        replica_groups=[list(range(NC))],
                        ins=[cc_ins[rep * L + l][:]], outs=[cc_outs[rep * L + l][:, :]])
                    ktf = work.tile([128, 2, N], F32, tag="ktf")
                    for a in range(2):
                        nc.sync.dma_start(
                            out=ktf[:, a].rearrange("(p) (c t) -> p c t", c=NC),
                            in_=cc_outs[rep * L + l].ap()[:, 0:KT_WORDS].rearrange(
                                "c (p a t) -> p a c t", p=128, a=2)[:, a])
                    vtiles = work.tile([128, NJT, NH, 33], BF16, tag="vtiles")
                    for c in range(NC):
                        nc.scalar.dma_start(
                            out=vtiles[T * (c % 2):T * (c % 2) + T, c // 2]
                                .rearrange("p a b -> p (a b)"),
                            in_=cc_outs[rep * L + l].ap()[c, KT_WORDS:CC_WORDS].bitcast(BF16)
                                .rearrange("(t r) -> t r", t=T))
                    if debug and rep == 0 and l == 0:
                        nc.sync.dma_start(out=dbg["kT"][:, :],
                                          in_=ktf.rearrange("p a t -> p (a t)"))

                    # ---- scores^T, exp, * expbias ----
                    probs = work.tile([128, NH, NJT, T], BF16, tag="probs")
                    for h in range(NH):
                        bk_ps = psSC.tile([128, NJT, T], F32, tag="sc")
                        for jt in range(NJT):
                            nc.tensor.matmul(
                                bk_ps[:, jt],
                                ktf[32 * (h % 4):32 * (h % 4) + 32, h // 4,
                                    128 * jt:128 * (jt + 1)],
                                qT[32 * (h % 4):32 * (h % 4) + 32, h // 4],
                                start=True, stop=True,
                                tile_position=(32 * (h % 4), 0))
                        nc.scalar.activation(
                            probs[:, h].rearrange("p j t -> p (j t)"),
                            bk_ps.rearrange("p j t -> p (j t)"), AF.Exp)
                        if debug and l == 0 and h == 0:
                            scf = work.tile([128, NJT * T], F32, tag="scdbg")
                            nc.vector.tensor_copy(
                                out=scf, in_=bk_ps.rearrange("p j t -> p (j t)"))
                            nc.sync.dma_start(out=dbg["sc0"][:, :], in_=scf)
                    nc.vector.tensor_tensor(
                        out=probs.rearrange("p h j t -> p (h j t)"),
                        in0=probs.rearrange("p h j t -> p (h j t)"),
                        in1=eb.rearrange("p h j t -> p (h j t)"), op=AL.mult)
                    if debug and rep == 0 and l == 0:
                        prf = work.tile([128, T], F32, tag="prf")
                        nc.vector.tensor_copy(out=prf, in_=probs[:, 0, 0])
                        nc.sync.dma_start(out=dbg["pr0"][:, :], in_=prf)

                    # ---- o = A@V (+ row sums via ones col), normalized ----
                    o_ps = psO.tile([64, NH, 33], F32, tag="o")
                    for h in range(NH):
                        for jt in range(NJT):
                            nc.tensor.matmul(o_ps[:, h], probs[:, h, jt],
                                             vtiles[:, jt, h],
                                             start=(jt == 0), stop=(jt == NJT - 1))
                    rec = small.tile([64, NH], F32, tag="rec")
                    nc.vector.reciprocal(out=rec, in_=o_ps[:, :, 32])
                    o_sb = work.tile([64, H], F32, tag="o_sb")
                    for h in range(NH):
                        nc.vector.tensor_scalar(
                            out=o_sb[:, 32 * h:32 * (h + 1)], in0=o_ps[:, h, 0:32],
                            scalar1=rec[:, h:h + 1], scalar2=None, op0=AL.mult)
                    if debug and rep == 0 and l == 0:
                        nc.sync.dma_start(out=dbg["o0"][:, :], in_=o_sb)

                    # ---- h += o @ Wo + bo ----
                    oT = work.tile([128, 2, T], F32, tag="oT")
                    for a in range(2):
                        tp = psMM.tile([128, T], F32, tag="mm")
                        nc.tensor.transpose(tp, o_sb[:, 128 * a:128 * (a + 1)],
                                            ident[:T, :T])
                        nc.scalar.activation(oT[:, a], tp, AF.Copy)
                    at_ps = psMM.tile([64, H], F32, tag="mm")
                    for a in range(2):
                        nc.tensor.matmul(at_ps, oT[:, a], wsb["Wo"][:, l, a],
                                         start=(a == 0), stop=(a == 1))
                    nc.vector.tensor_tensor(out=h_sb[:T], in0=h_sb[:T], in1=at_ps,
                                            op=AL.add)
                    nc.vector.tensor_tensor(out=h_sb[:T], in0=h_sb[:T],
                                            in1=bm_r[:T, 1], op=AL.add)
                    if debug and rep == 0 and l == 0:
                        nc.sync.dma_start(out=dbg["h1"][:, :], in_=h_sb[:T])

                    # ---- LN2 + FFN ----
                    mv2, rstd2 = layernorm_stats(h_sb[:T], f"ln2")
                    y2 = work.tile([128, H], F32, tag="y2")
                    nc.vector.tensor_scalar(out=y2[:T], in0=h_sb[:T],
                                            scalar1=mv2[:T, 0:1], scalar2=rstd2[:T],
                                            op0=AL.subtract, op1=AL.mult)
                    y2T = work.tile([128, 2, T], F32, tag="y2T")
                    for a in range(2):
                        tp = psMM.tile([128, T], F32, tag="mm")
                        nc.tensor.transpose(tp, y2[:T, 128 * a:128 * (a + 1)],
                                            ident[:T, :T])
                        nc.scalar.activation(y2T[:, a], tp, AF.Identity,
                                             bias=bsb["ln2_b"][:, l, a:a + 1],
                                             scale=bsb["ln2_s"][:, l, a:a + 1])
                    z_ps = psMM.tile([64, H], F32, tag="mm")
                    for a in range(2):
                        nc.tensor.matmul(z_ps, y2T[:, a], wsb["W1"][:, l, a],
                                         start=(a == 0), stop=(a == 1))
                    z = work.tile([64, H], F32, tag="z")
                    nc.vector.tensor_tensor(out=z, in0=z_ps, in1=bm_r[:T, 2],
                                            op=AL.add)
                    # gelu(z), tanh approx (same ACT table set as exp)
                    z2 = work.tile([64, H], F32, tag="z2")
                    nc.scalar.activation(z2, z, AF.Square)
                    gw = work.tile([64, H], F32, tag="gw")
                    nc.scalar.activation(gw, z2, AF.Copy, bias=GC1, scale=GC2)
                    gu = work.tile([64, H], F32, tag="gu")
                    nc.vector.tensor_tensor(out=gu, in0=gw, in1=z, op=AL.mult)
                    gt = work.tile([64, H], F32, tag="gt")
                    nc.scalar.activation(gt, gu, AF.Tanh)
                    nc.scalar.activation(gt, gt, AF.Copy, bias=0.5, scale=0.5)
                    gg = work.tile([64, H], F32, tag="gg")
                    nc.vector.tensor_tensor(out=gg, in0=gt, in1=z, op=AL.mult)
                    gT = work.tile([128, 2, T], F32, tag="gT")
                    for a in range(2):
                        tp = psMM.tile([128, T], F32, tag="mm")
                        nc.tensor.transpose(tp, gg[:, 128 * a:128 * (a + 1)],
                                            ident[:T, :T])
                        nc.scalar.activation(gT[:, a], tp, AF.Copy)
                    f_ps = psMM.tile([64, H], F32, tag="mm")
                    for a in range(2):
                        nc.tensor.matmul(f_ps, gT[:, a], wsb["W2"][:, l, a],
                                         start=(a == 0), stop=(a == 1))
                    nc.vector.tensor_tensor(out=h_sb[:T], in0=h_sb[:T], in1=f_ps,
                                            op=AL.add)
                    nc.vector.tensor_tensor(out=h_sb[:T], in0=h_sb[:T],
                                            in1=bm_r[:T, 3], op=AL.add)

                nc.sync.dma_start(out=out_t[:, :], in_=h_sb[:T])
                bctx2.close()

    nc.compile()
    return nc


# ---------------- host marshalling (reshape/cast only) ----------------

def _prep_inputs(inputs):
    import ml_dtypes

    def f32(a):
        return np.ascontiguousarray(np.asarray(a, np.float32))

    x = f32(inputs["x"])
    ee = np.asarray(inputs["edge_encodes"]).astype(np.int64)
    ede = np.asarray(inputs["edge_dist_encodes"]).astype(np.int64)[:, :, 0]

    shared = {
        "w_feat": f32(inputs["W_feat"]).reshape(2, 128, H),
        "b_feat": f32(inputs["b_feat"]),
        "edge_emb": f32(inputs["edge_emb"]),
        "edge_embT": f32(np.asarray(inputs["edge_emb"], np.float32).T
                         .reshape(2, 128, EEN_W)),
        "edge_dist_emb": f32(inputs["edge_dist_emb"]),
        "edge_dist_embT": f32(np.asarray(inputs["edge_dist_emb"], np.float32).T
                              .reshape(2, 128, EDN)),
        "w_ee": f32(inputs["W_ee"]).reshape(2, 128, NH),
        "w_ed": f32(inputs["W_ed"]).reshape(2, 128, NH),
        # table-eviction partition p = 4h+g -> head h = p//4
        "bee32": np.repeat(f32(inputs["b_ee"]), 4),
        "bed32": np.repeat(f32(inputs["b_ed"]), 4),
        "ident": np.eye(128, dtype=np.float32),
        "iota16": (np.arange(128) % 16).astype(np.float32),
        "iota32": (np.arange(128) % 32).astype(np.float32),
    }
    for n in ["Wq", "Wk", "Wv", "Wo", "W1", "W2"]:
        shared[n] = f32(inputs[n]).reshape(L, 2, 128, H)
    for n in ["bq", "bk", "bv", "bo", "b1", "b2", "ln1_s", "ln1_b",
              "ln2_s", "ln2_b"]:
        shared[n] = f32(inputs[n])
    shared["bmisc"] = np.ascontiguousarray(np.stack(
        [shared["bv"], shared["bo"], shared["b1"], shared["b2"]],
        axis=1))

    in_maps = []
    for c in range(NC):
        rows = slice(T * c, T * (c + 1))
        m = dict(shared)
        m["xT"] = np.ascontiguousarray(x[rows].T).reshape(2, 128, T)
        # t2: [4 g, (jt jj i)] where j = jt*128 + g*32 + jj
        e2 = ede[rows].T.astype(np.float32).reshape(NJT, 4, 32, T)
        m["t2"] = np.ascontiguousarray(
            e2.transpose(1, 2, 0, 3).reshape(4, 8192)).astype(ml_dtypes.bfloat16)
        # t1 per w: [8 g, (jj jt i)] where j = jt*128 + g*16 + jj
        for w, nm in [(0, "t1a"), (1, "t1b")]:
            e1 = ee[rows, :, w].T.astype(np.float32).reshape(NJT, 8, 16, T)
            m[nm] = np.ascontiguousarray(
                e1.transpose(1, 2, 0, 3).reshape(8, 4096)).astype(ml_dtypes.bfloat16)
        in_maps.append(m)
    return in_maps


def kernel(**inputs):
    debug = inputs.pop("_debug", False)
    trace = inputs.pop("_trace", False)
    tmpdir = inputs.pop("_tmpdir", None)
    key = ("k", debug)
    if key not in _CACHE:
        _CACHE[key] = build(debug=debug)
    nc = _CACHE[key]
    in_maps = _prep_inputs(inputs)
    res = run_bass_kernel_spmd(nc, in_maps, list(range(NC)), trace=trace,
                               tmpdir=tmpdir)
    kernel._last = res
    out = np.concatenate([res.results[c]["out"] for c in range(NC)], axis=0)
    if debug:
        kernel._dbg = [{k[4:]: v for k, v in res.results[c].items()
                        if k.startswith("dbg_")} for c in range(NC)]
    return out



# revision 12
# speedup vs baseline: 1.8225x; 1.8225x over previous
"""DyGraphTransformer forward on 8 trn2 NeuronCores (Bass/Tile), v2.

Sequence-parallel over N=512 rows (64 per core).  Per layer, the post-LN1
activations y^T (bf16, 32KB) are AllGathered; each core then computes full
K/V locally from the gathered y (replicated weights), so the collective is
small and kicks off right after LN1.

The Graphormer bias is applied as exp(bias) multiplied into exp(scores).
The two tiny embedding tables are renormed+projected+exponentiated on the
host (pure weight preprocessing); the per-(i,j) gather runs on device via
block-diagonal one-hot matmuls (8 j-groups x 8 heads per pass), with the
three gathered tables multiplied together on DVE.

All heavy matmuls run in bf16 (fp32 PSUM accumulate); the residual stream
stays fp32.  ACT stays on the exp_and_others table set (exp/tanh/copies);
LN rsqrt is a DVE bit-trick + Newton.
"""

import sys

sys.path.insert(0, "/opt/trn_rl_repo")

import contextlib

import numpy as np

import concourse.bacc as bacc
import concourse.bass as bass
import concourse.tile as tile
from concourse import mybir
from concourse.bass_utils import run_bass_kernel_spmd

# model dims
N, F, H, NH, L, W = 512, 256, 256, 8, 6, 2
DK = H // NH                 # 32
NC = 8                       # cores
T = N // NC                  # 64 tokens per core
NJT = N // 128               # 4 j-tiles
LN_EPS = 1e-5
SCALE = DK ** -0.5

F32 = mybir.dt.float32
BF16 = mybir.dt.bfloat16
I32 = mybir.dt.int32
AL = mybir.AluOpType
AF = mybir.ActivationFunctionType

GC1 = 0.7978845608028654     # sqrt(2/pi)
GC2 = GC1 * 0.044715
SQ_GC2 = GC2 ** 0.5

NPASS = {"t1a": 2, "t1b": 2, "t2": 8}   # 16-entry subtables

_CACHE = {}


def _gbcast(ap, rep, ncols):
    """AP [G, ncols] -> [G*rep partitions, ncols], each row replicated."""
    g = ap.ap[0][1]
    return bass.AP(tensor=ap.tensor, offset=ap.offset,
                   ap=[[ap.ap[0][0], g], [0, rep], [1, ncols]])


def _bcast_row(dram_ap, p):
    """1-D DRAM AP [Hf] -> broadcast AP [p, Hf]."""
    return bass.AP(tensor=dram_ap.tensor, offset=dram_ap.offset,
                   ap=[[0, p]] + [list(x) for x in dram_ap.ap])


def build(debug=False):
    nc = bacc.Bacc("TRN2", target_bir_lowering=False, debug=False,
                   num_devices=NC)

    # ---------------- DRAM I/O ----------------
    xT_in = nc.dram_tensor("xT", [2, 128, T], F32, kind="ExternalInput")
    wfeat_in = nc.dram_tensor("w_feat", [2, 128, H], F32, kind="ExternalInput")
    bfeat_in = nc.dram_tensor("b_feat", [H], F32, kind="ExternalInput")
    ident_in = nc.dram_tensor("identbf", [128, 128], BF16, kind="ExternalInput")
    iota16_in = nc.dram_tensor("iota16", [128], F32, kind="ExternalInput")
    # 10 block-diag lhsT tables: [0:2]=t1 passes, [2:10]=t2 passes
    eblhs_in = nc.dram_tensor("eb_lhs", [10, 128, 64], BF16, kind="ExternalInput")
    # gather indices, bf16 values, [3 tabs, 8 g, 4096 (jj jt i)]
    idx_in = nc.dram_tensor("idx3", [3, 8, 4096], BF16, kind="ExternalInput")

    w_names = ["Wq", "Wk", "Wv", "Wo", "W1", "W2"]
    w_ins = {n: nc.dram_tensor(n, [L, 2, 128, H], BF16, kind="ExternalInput")
             for n in w_names}
    b_names = ["bq", "bk", "ln1_s", "ln1_b", "ln2_s", "ln2_b", "bv"]
    b_ins = {n: nc.dram_tensor(n, [L, H], F32, kind="ExternalInput")
             for n in b_names}
    # row-broadcast biases (per free dim): bo, b1, b2
    brow_in = nc.dram_tensor("brow", [L, 3, H], F32, kind="ExternalInput")

    out_t = nc.dram_tensor("out", [T, H], F32, kind="ExternalOutput")

    CCW = H * T                   # 16384 bf16 = 32KB
    cc_ins = [nc.dram_tensor(f"cc_in{i}", [CCW], BF16) for i in range(L)]
    cc_outs = [nc.dram_tensor(f"cc_out{i}", [NC, CCW], BF16,
                              addr_space="Shared") for i in range(L)]

    with tile.TileContext(nc) as tc:
        ctx = contextlib.ExitStack()
        with ctx:
            const = ctx.enter_context(tc.tile_pool(name="const", bufs=1))
            wpool = ctx.enter_context(tc.tile_pool(name="weights", bufs=1))
            small = ctx.enter_context(tc.tile_pool(name="small", bufs=2))
            psT = ctx.enter_context(tc.tile_pool(name="psT", bufs=2, space="PSUM"))
            psTP = ctx.enter_context(tc.tile_pool(name="psTP", bufs=2, space="PSUM"))

            # ---------------- constants / weights ----------------
            ident = const.tile([128, 128], BF16)
            nc.sync.dma_start(out=ident, in_=ident_in[:, :])
            iota16f = const.tile([128, 1], F32)
            nc.scalar.dma_start(out=iota16f,
                                in_=iota16_in.ap().rearrange("(p o) -> p o", o=1))
            magic = const.tile([128, 1], I32)
            nc.vector.memset(magic, 0x5F3759DF)

            wfeat = const.tile([128, 2, H], F32)
            nc.sync.dma_start(out=wfeat,
                              in_=wfeat_in.ap().rearrange("a p f -> p a f"))
            bfeat_r = const.tile([64, H], F32)
            nc.scalar.dma_start(out=bfeat_r, in_=_bcast_row(bfeat_in.ap(), T))
            xT = const.tile([128, 2, T], F32)
            nc.sync.dma_start(out=xT, in_=xT_in.ap().rearrange("a p t -> p a t"))

            wsb = {}
            for i, n in enumerate(w_names):
                tl = wpool.tile([128, L, 2, H], BF16, tag="w_" + n)
                eng = nc.sync if i % 2 == 0 else nc.scalar
                eng.dma_start(out=tl,
                              in_=w_ins[n].ap().rearrange("l a p f -> p l a f"))
                wsb[n] = tl
            bsb = {}
            for i, n in enumerate(b_names):
                tl = wpool.tile([128, L, 2], F32, tag="b_" + n)
                eng = nc.sync if i % 2 == 0 else nc.scalar
                eng.dma_start(
                    out=tl, in_=b_ins[n].ap().rearrange("l (a p) -> p l a", p=128))
                bsb[n] = tl
            # row-broadcast biases: [64, L, 3, H]
            brow = wpool.tile([64, L, 3, H], F32, tag="brow")
            nc.scalar.dma_start(out=brow, in_=_bcast_row(
                brow_in.ap().rearrange("l k f -> (l k f)"), T).rearrange(
                    "p (l k f) -> p l k f", l=L, k=3))
            eblhs = const.tile([128, 10, 64], BF16)
            nc.sync.dma_start(out=eblhs,
                              in_=eblhs_in.ap().rearrange("k p c -> p k c"))

            # ---------------- helpers ----------------
            def rsqrt_col(u_ap, p, tagp, iters=2):
                ki = small.tile([128, 1], I32, tag=tagp + "ki")
                nc.vector.tensor_scalar(out=ki[:p], in0=u_ap.bitcast(I32),
                                        scalar1=1, scalar2=None,
                                        op0=AL.logical_shift_right)
                z = small.tile([128, 1], F32, tag=tagp + "z")
                nc.vector.tensor_tensor(out=z[:p].bitcast(I32), in0=magic[:p],
                                        in1=ki[:p], op=AL.subtract)
                t = small.tile([128, 1], F32, tag=tagp + "t")
                for _ in range(iters):
                    nc.vector.tensor_scalar(out=t[:p], in0=z[:p], scalar1=z[:p],
                                            scalar2=u_ap, op0=AL.mult, op1=AL.mult)
                    nc.vector.tensor_scalar(out=t[:p], in0=t[:p], scalar1=-0.5,
                                            scalar2=1.5, op0=AL.mult, op1=AL.add)
                    nc.vector.tensor_tensor(out=z[:p], in0=z[:p], in1=t[:p],
                                            op=AL.mult)
                return z

            def layernorm_stats(h_ap, tagp):
                stats = small.tile([128, 6], F32, tag=tagp + "st")
                nc.vector.bn_stats(out=stats[:T], in_=h_ap)
                mv = small.tile([128, 2], F32, tag=tagp + "mv")
                nc.vector.bn_aggr(out=mv[:T], in_=stats[:T])
                u = small.tile([128, 1], F32, tag=tagp + "u")
                nc.vector.tensor_scalar(out=u[:T], in0=mv[:T, 1:2],
                                        scalar1=LN_EPS, scalar2=None, op0=AL.add)
                rstd = rsqrt_col(u[:T], T, tagp)
                return mv, rstd

            # =====================================================
            # Stage A: h0 = x @ Wfeat + b (f32); layer-0 LN1 -> yT -> send
            # =====================================================
            h_sb = const.tile([64, H], F32, tag="resid")
            h_ps = psT.tile([64, H], F32, tag="mm")
            for a in range(2):
                nc.tensor.matmul(h_ps, xT[:, a], wfeat[:, a],
                                 start=(a == 0), stop=(a == 1))
            nc.vector.tensor_tensor(out=h_sb, in0=h_ps, in1=bfeat_r,
                                    op=AL.add)

            yT_sb = const.tile([128, 2, T], BF16, tag="yT")
            y0 = const.tile([64, H], BF16, tag="y0")

            def ln1_to_yT(l):
                mv, rstd = layernorm_stats(h_sb, "ln1")
                nc.vector.tensor_scalar(out=y0, in0=h_sb,
                                        scalar1=mv[:T, 0:1], scalar2=rstd[:T],
                                        op0=AL.subtract, op1=AL.mult)
                for a in range(2):
                    tp = psTP.tile([128, T], BF16, tag="tp")
                    nc.tensor.transpose(tp, y0[:, 128 * a:128 * (a + 1)],
                                        ident[:T, :T])
                    nc.scalar.activation(yT_sb[:, a], tp, AF.Identity,
                                         bias=bsb["ln1_b"][:, l, a:a + 1],
                                         scale=bsb["ln1_s"][:, l, a:a + 1])
                nc.sync.dma_start(
                    out=cc_ins[l].ap().rearrange("(p c) -> p c", p=128),
                    in_=yT_sb.rearrange("p a t -> p (a t)"))
                nc.gpsimd.collective_compute(
                    "AllGather", AL.bypass,
                    replica_groups=[list(range(NC))],
                    ins=[cc_ins[l][:]], outs=[cc_outs[l][:, :]])

            ln1_to_yT(0)

            # =====================================================
            # Stage B: bias gather preamble (overlaps barrier + AG0)
            # =====================================================
            bctx = contextlib.ExitStack()
            bb = bctx.enter_context(tc.tile_pool(name="biasbuild", bufs=1))
            ohp = bctx.enter_context(tc.tile_pool(name="ohp", bufs=2))
            psG = bctx.enter_context(tc.tile_pool(name="psG", bufs=2, space="PSUM"))

            idx_t = {}
            for k, tab in enumerate(["t1a", "t1b", "t2"]):
                it = bb.tile([128, 4096], BF16, tag="idx_" + tab)
                eng = nc.sync if k % 2 == 0 else nc.scalar
                eng.dma_start(out=it, in_=_gbcast(idx_in.ap()[k], 16, 4096))
                idx_t[tab] = it

            hm = {tab: bb.tile([64, 4096], BF16, tag="hm_" + tab,
                               name="hm_" + tab)
                  for tab in ["t1a", "t1b", "t2"]}

            for tab, lhs0 in [("t1a", 0), ("t1b", 0), ("t2", 2)]:
                npass = NPASS[tab]
                for ch in range(4):              # 1024-col chunks
                    ps = psG.tile([64, 1024], F32, tag="g")
                    for q in range(npass):
                        oh = ohp.tile([128, 1024], BF16, tag="oh")
                        nc.vector.tensor_scalar(
                            out=oh, in0=idx_t[tab][:, 1024 * ch:1024 * (ch + 1)],
                            scalar1=float(16 * q), scalar2=iota16f,
                            op0=AL.subtract, op1=AL.is_equal)
                        for hf in range(2):
                            nc.tensor.matmul(
                                ps[:, 512 * hf:512 * (hf + 1)],
                                eblhs[:, lhs0 + q],
                                oh[:, 512 * hf:512 * (hf + 1)],
                                start=(q == 0), stop=(q == npass - 1))
                    nc.scalar.activation(hm[tab][:, 1024 * ch:1024 * (ch + 1)],
                                         ps, AF.Copy)

            # prod = t1a * t1b * t2  (gather layout [64=(8h+g), 4096])
            t12 = bb.tile([64, 4096], BF16, tag="t12")
            nc.vector.tensor_tensor(out=t12, in0=hm["t1a"], in1=hm["t1b"],
                                    op=AL.mult)
            prod = bb.tile([64, 4096], BF16, tag="prod")
            nc.vector.tensor_tensor(out=prod, in0=t12, in1=hm["t2"],
                                    op=AL.mult)

            # reorient to eb [128 j, (m, jt, h', i)]
            eb = const.tile([128, 2, NJT, 4, T], BF16, tag="eb")
            for h in range(NH):
                m, hp = h // 4, h % 4
                eng = nc.sync if h % 2 == 0 else nc.scalar
                eng.dma_start(
                    out=eb[:, m, :, hp, :],
                    in_=prod[8 * h:8 * h + 8].rearrange(
                        "g (jj r) -> g jj r", jj=16))

            bctx.close()

            # =====================================================
            # Stage C: layers
            # =====================================================
            lctx = contextlib.ExitStack()
            work = lctx.enter_context(tc.tile_pool(name="work", bufs=2))
            psS = lctx.enter_context(tc.tile_pool(name="psS", bufs=2, space="PSUM"))

            # persistent attention tiles
            qbd = const.tile([128, 2, 4 * T], BF16, tag="qbd")
            nc.vector.memset(qbd.rearrange("p a c -> p (a c)"), 0.0)
            vt = const.tile([128, NJT, NH, 33], BF16, tag="vt")
            nc.vector.memset(vt.rearrange("p j h d -> p (j h d)"), 1.0)

            for l in range(L):
                if l > 0:
                    ln1_to_yT(l)

                # ---- q block-diag (overlaps AG) ----
                q_ps = psT.tile([128, 2, T], F32, tag="mm")
                for m2 in range(2):
                    for a in range(2):
                        nc.tensor.matmul(
                            q_ps[:, m2],
                            wsb["Wq"][:, l, a, 128 * m2:128 * (m2 + 1)],
                            yT_sb[:, a], start=(a == 0), stop=(a == 1))
                for h in range(NH):
                    m2, hp = h // 4, h % 4
                    nc.vector.tensor_scalar(
                        out=qbd[32 * hp:32 * hp + 32, m2,
                                T * hp:T * hp + T],
                        in0=q_ps[32 * hp:32 * hp + 32, m2],
                        scalar1=bsb["bq"][32 * hp:32 * hp + 32, l,
                                          m2:m2 + 1],
                        scalar2=None, op0=AL.add)

                # ---- AG lands: read back gathered yT ----
                ygT = work.tile([128, 2, N], BF16, tag="ygT")
                for a in range(2):
                    eng = nc.sync if a == 0 else nc.scalar
                    eng.dma_start(
                        out=ygT[:, a].rearrange("p (c t) -> p c t", c=NC),
                        in_=cc_outs[l].ap().rearrange(
                            "c (p a t) -> p a c t", p=128, a=2)[:, a])

                # ---- full K^T ----
                kT = work.tile([128, 2, N], BF16, tag="kT")
                for m2 in range(2):
                    k_ps = psT.tile([128, N], F32, tag="mm")
                    for a in range(2):
                        nc.tensor.matmul(
                            k_ps, wsb["Wk"][:, l, a, 128 * m2:128 * (m2 + 1)],
                            ygT[:, a], start=(a == 0), stop=(a == 1))
                    nc.vector.tensor_scalar(
                        out=kT[:, m2], in0=k_ps,
                        scalar1=bsb["bk"][:, l, m2:m2 + 1],
                        scalar2=None, op0=AL.add)

                # ---- full V (token-major per j-tile), ones col persists ----
                for jt in range(NJT):
                    v_ps = psT.tile([128, H], F32, tag="mm")
                    for a in range(2):
                        nc.tensor.matmul(
                            v_ps, ygT[:, a, 128 * jt:128 * (jt + 1)],
                            wsb["Wv"][:, l, a], start=(a == 0), stop=(a == 1))
                    nc.vector.tensor_copy(
                        out=vt[:, jt, :, 0:32],
                        in_=v_ps.rearrange("p (h d) -> p h d", h=NH))

                # ---- scores + exp + bias-mult ----
                probs = work.tile([128, 2, NJT, 4, T], BF16, tag="probs")
                for m2 in range(2):
                    s_ps = psS.tile([128, NJT, 4, T], F32, tag="sc")
                    for jt in range(NJT):
                        nc.tensor.matmul(
                            s_ps[:, jt],
                            kT[:, m2, 128 * jt:128 * (jt + 1)],
                            qbd[:, m2], start=True, stop=True)
                    nc.scalar.activation(
                        probs[:, m2].rearrange("p j h t -> p (j h t)"),
                        s_ps.rearrange("p j h t -> p (j h t)"), AF.Exp)
                nc.vector.tensor_tensor(
                    out=probs.rearrange("p m j h t -> p (m j h t)"),
                    in0=probs.rearrange("p m j h t -> p (m j h t)"),
                    in1=eb.rearrange("p m j h t -> p (m j h t)"), op=AL.mult)

                # ---- A@V with ones-col row sums ----
                o_ps = psT.tile([64, NH, 33], F32, tag="mm")
                for h in range(NH):
                    m2, hp = h // 4, h % 4
                    for jt in range(NJT):
                        nc.tensor.matmul(o_ps[:, h], probs[:, m2, jt, hp],
                                         vt[:, jt, h],
                                         start=(jt == 0), stop=(jt == NJT - 1))
                rec = small.tile([64, NH], F32, tag="rec")
                nc.vector.reciprocal(out=rec, in_=o_ps[:, :, 32])
                o_sb = work.tile([64, H], BF16, tag="o_sb")
                for h in range(NH):
                    nc.vector.tensor_scalar(
                        out=o_sb[:, 32 * h:32 * (h + 1)], in0=o_ps[:, h, 0:32],
                        scalar1=rec[:, h:h + 1], scalar2=None, op0=AL.mult)

                # ---- h += (o + bv-fold) @ Wo + bo ----
                oT = work.tile([128, 2, T], BF16, tag="oT")
                for a in range(2):
                    tp = psTP.tile([128, T], BF16, tag="tp")
                    nc.tensor.transpose(tp, o_sb[:, 128 * a:128 * (a + 1)],
                                        ident[:T, :T])
                    nc.scalar.activation(oT[:, a], tp, AF.Identity,
                                         bias=bsb["bv"][:, l, a:a + 1],
                                         scale=1.0)
                at_ps = psT.tile([64, H], F32, tag="mm")
                for a in range(2):
                    nc.tensor.matmul(at_ps, oT[:, a], wsb["Wo"][:, l, a],
                                     start=(a == 0), stop=(a == 1))
                nc.vector.tensor_tensor(out=h_sb, in0=h_sb, in1=at_ps,
                                        op=AL.add)
                nc.vector.tensor_tensor(out=h_sb, in0=h_sb,
                                        in1=brow[:, l, 0], op=AL.add)

                # ---- LN2 + FFN ----
                mv2, rstd2 = layernorm_stats(h_sb, "ln2")
                y2 = work.tile([64, H], BF16, tag="y2")
                nc.vector.tensor_scalar(out=y2, in0=h_sb,
                                        scalar1=mv2[:T, 0:1], scalar2=rstd2[:T],
                                        op0=AL.subtract, op1=AL.mult)
                y2T = work.tile([128, 2, T], BF16, tag="y2T")
                for a in range(2):
                    tp = psTP.tile([128, T], BF16, tag="tp")
                    nc.tensor.transpose(tp, y2[:, 128 * a:128 * (a + 1)],
                                        ident[:T, :T])
                    nc.scalar.activation(y2T[:, a], tp, AF.Identity,
                                         bias=bsb["ln2_b"][:, l, a:a + 1],
                                         scale=bsb["ln2_s"][:, l, a:a + 1])
                z_ps = psT.tile([64, H], F32, tag="mm")
                for a in range(2):
                    nc.tensor.matmul(z_ps, y2T[:, a], wsb["W1"][:, l, a],
                                     start=(a == 0), stop=(a == 1))
                z_sb = work.tile([64, H], BF16, tag="z")
                nc.vector.tensor_tensor(out=z_sb, in0=z_ps, in1=brow[:, l, 1],
                                        op=AL.add)
                # tanh-gelu: gg = z * (0.5 + 0.5*tanh(z*(GC1 + GC2 z^2)))
                z2 = work.tile([64, H], BF16, tag="z2")
                nc.scalar.activation(z2, z_sb, AF.Square, scale=SQ_GC2)
                zg = work.tile([64, H], BF16, tag="zg")
                nc.vector.tensor_scalar(out=zg, in0=z2, scalar1=GC1,
                                        scalar2=None, op0=AL.add)
                gu = work.tile([64, H], BF16, tag="gu")
                nc.vector.tensor_tensor(out=gu, in0=zg, in1=z_sb, op=AL.mult)
                gt = work.tile([64, H], BF16, tag="gt")
                nc.scalar.activation(gt, gu, AF.Tanh)
                gh = work.tile([64, H], BF16, tag="gh")
                nc.scalar.activation(gh, gt, AF.Copy, bias=0.5, scale=0.5)
                gg = work.tile([64, H], BF16, tag="gg")
                nc.vector.tensor_tensor(out=gg, in0=gh, in1=z_sb, op=AL.mult)
                gT = work.tile([128, 2, T], BF16, tag="gT")
                for a in range(2):
                    tp = psTP.tile([128, T], BF16, tag="tp")
                    nc.tensor.transpose(tp, gg[:, 128 * a:128 * (a + 1)],
                                        ident[:T, :T])
                    nc.vector.tensor_copy(out=gT[:, a], in_=tp)
                f_ps = psT.tile([64, H], F32, tag="mm")
                for a in range(2):
                    nc.tensor.matmul(f_ps, gT[:, a], wsb["W2"][:, l, a],
                                     start=(a == 0), stop=(a == 1))
                nc.vector.tensor_tensor(out=h_sb, in0=h_sb, in1=f_ps,
                                        op=AL.add)
                nc.vector.tensor_tensor(out=h_sb, in0=h_sb,
                                        in1=brow[:, l, 2], op=AL.add)

            nc.sync.dma_start(out=out_t[:, :], in_=h_sb)
            lctx.close()

    nc.compile()
    return nc


# ---------------- host marshalling ----------------

def _prep_inputs(inputs):
    import ml_dtypes

    BF = ml_dtypes.bfloat16

    def f32(a):
        return np.ascontiguousarray(np.asarray(a, np.float32))

    def bf16(a):
        return np.ascontiguousarray(np.asarray(a).astype(BF))

    x = f32(inputs["x"])
    ee = np.asarray(inputs["edge_encodes"]).astype(np.int64)
    ede = np.asarray(inputs["edge_dist_encodes"]).astype(np.int64)[:, :, 0]

    # --- weight preprocessing: renorm + project + exp the bias tables ---
    def renorm(t):
        t = np.asarray(t, np.float64)
        n = np.linalg.norm(t, axis=-1, keepdims=True)
        return t * np.where(n > 1.0, 1.0 / (n + 1e-7), 1.0)

    p1 = renorm(inputs["edge_emb"]) @ np.asarray(inputs["W_ee"], np.float64)
    p2 = renorm(inputs["edge_dist_emb"]) @ np.asarray(inputs["W_ed"], np.float64)
    t1exp = np.exp(0.5 * p1)                                   # [32, 8]
    t2exp = np.exp(p2 + np.asarray(inputs["b_ee"], np.float64)
                   + np.asarray(inputs["b_ed"], np.float64))   # [128, 8]

    # block-diag lhsT tiles [10, 128, 64]: [q][16g+e', 8h+g]
    eb_lhs = np.zeros((10, 8, 16, 8, 8), np.float64)
    for q in range(2):
        for g in range(8):
            eb_lhs[q, g, :, :, g] = t1exp[16 * q:16 * q + 16]
    for q in range(8):
        for g in range(8):
            eb_lhs[2 + q, g, :, :, g] = t2exp[16 * q:16 * q + 16]
    eb_lhs = eb_lhs.reshape(10, 128, 64)

    shared = {
        "w_feat": f32(inputs["W_feat"]).reshape(2, 128, H),
        "b_feat": f32(inputs["b_feat"]),
        "identbf": np.eye(128, dtype=np.float32).astype(BF),
        "iota16": (np.arange(128) % 16).astype(np.float32),
        "eb_lhs": eb_lhs.astype(BF),
        "bq": f32(np.asarray(inputs["bq"], np.float64) * SCALE),
        "bk": f32(inputs["bk"]),
        "bv": f32(inputs["bv"]),
        "ln1_s": f32(inputs["ln1_s"]), "ln1_b": f32(inputs["ln1_b"]),
        "ln2_s": f32(inputs["ln2_s"]), "ln2_b": f32(inputs["ln2_b"]),
        "Wq": bf16(np.asarray(inputs["Wq"], np.float64) * SCALE)
            .reshape(L, 2, 128, H),
    }
    for n in ["Wk", "Wv", "Wo", "W1", "W2"]:
        shared[n] = bf16(inputs[n]).reshape(L, 2, 128, H)
    shared["brow"] = np.ascontiguousarray(np.stack(
        [f32(inputs["bo"]), f32(inputs["b1"]), f32(inputs["b2"])],
        axis=1))

    in_maps = []
    for c in range(NC):
        rows = slice(T * c, T * (c + 1))
        m = dict(shared)
        m["xT"] = np.ascontiguousarray(x[rows].T).reshape(2, 128, T)
        # idx layout [tab, 8 g, (16 jj, 4 jt, 64 i)], j = jt*128 + g*16 + jj
        idx3 = np.empty((3, 8, 16, NJT, T), np.float32)
        for k, arr in enumerate([ee[rows, :, 0], ee[rows, :, 1], ede[rows]]):
            # arr [64 i, 512 j] -> [jt, g, jj, i] -> [g, jj, jt, i]
            a4 = arr.T.reshape(NJT, 8, 16, T).transpose(1, 2, 0, 3)
            idx3[k] = a4
        m["idx3"] = np.ascontiguousarray(idx3.reshape(3, 8, 4096)).astype(BF)
        in_maps.append(m)
    return in_maps


def kernel(**inputs):
    debug = inputs.pop("_debug", False)
    trace = inputs.pop("_trace", False)
    tmpdir = inputs.pop("_tmpdir", None)
    key = ("k", debug)
    if key not in _CACHE:
        _CACHE[key] = build(debug=debug)
    nc = _CACHE[key]
    in_maps = _prep_inputs(inputs)
    res = run_bass_kernel_spmd(nc, in_maps, list(range(NC)), trace=trace,
                               tmpdir=tmpdir)
    kernel._last = res
    out = np.concatenate([res.results[c]["out"] for c in range(NC)], axis=0)
    return out


# revision 17
# speedup vs baseline: 2.0192x; 1.1079x over previous
"""DyGraphTransformer forward on 8 trn2 NeuronCores (Bass/Tile), v2.

Sequence-parallel over N=512 rows (64 per core).  Per layer, the post-LN1
activations y^T (bf16, 32KB) are AllGathered; each core then computes full
K/V locally from the gathered y (replicated weights), so the collective is
small and kicks off right after LN1.

The Graphormer bias is applied as exp(bias) multiplied into exp(scores).
The two tiny embedding tables are renormed+projected+exponentiated on the
host (pure weight preprocessing); the per-(i,j) gather runs on device via
block-diagonal one-hot matmuls (8 j-groups x 8 heads per pass), with the
three gathered tables multiplied together on DVE.

All heavy matmuls run in bf16 (fp32 PSUM accumulate); the residual stream
stays fp32.  ACT stays on the exp_and_others table set (exp/tanh/copies);
LN rsqrt is a DVE bit-trick + Newton.
"""

import sys

sys.path.insert(0, "/opt/trn_rl_repo")

import contextlib

import numpy as np

import concourse.bacc as bacc
import concourse.bass as bass
import concourse.tile as tile
from concourse import mybir
from concourse.bass_utils import run_bass_kernel_spmd

# model dims
N, F, H, NH, L, W = 512, 256, 256, 8, 6, 2
DK = H // NH                 # 32
NC = 8                       # cores
T = N // NC                  # 64 tokens per core
NJT = N // 128               # 4 j-tiles
LN_EPS = 1e-5
SCALE = DK ** -0.5

F32 = mybir.dt.float32
BF16 = mybir.dt.bfloat16
I32 = mybir.dt.int32
AL = mybir.AluOpType
AF = mybir.ActivationFunctionType

GC1 = 0.7978845608028654     # sqrt(2/pi)
GC2 = GC1 * 0.044715
SQ_GC2 = GC2 ** 0.5

NPASS = {"t1a": 2, "t1b": 2, "t2": 8}   # 16-entry subtables

_CACHE = {}


def _gbcast(ap, rep, ncols):
    """AP [G, ncols] -> [G*rep partitions, ncols], each row replicated."""
    g = ap.ap[0][1]
    return bass.AP(tensor=ap.tensor, offset=ap.offset,
                   ap=[[ap.ap[0][0], g], [0, rep], [1, ncols]])


def _bcast_row(dram_ap, p):
    """1-D DRAM AP [Hf] -> broadcast AP [p, Hf]."""
    return bass.AP(tensor=dram_ap.tensor, offset=dram_ap.offset,
                   ap=[[0, p]] + [list(x) for x in dram_ap.ap])


def build(debug=False):
    nc = bacc.Bacc("TRN2", target_bir_lowering=False, debug=False,
                   num_devices=NC)

    # ---------------- DRAM I/O ----------------
    xT_in = nc.dram_tensor("xT", [2, 128, T], F32, kind="ExternalInput")
    wfeat_in = nc.dram_tensor("w_feat", [2, 128, H], F32, kind="ExternalInput")
    bfeat_in = nc.dram_tensor("b_feat", [H], F32, kind="ExternalInput")
    ident_in = nc.dram_tensor("identbf", [128, 128], BF16, kind="ExternalInput")
    iota16_in = nc.dram_tensor("iota16", [128], F32, kind="ExternalInput")
    # 10 block-diag lhsT tables: [0:2]=t1 passes, [2:10]=t2 passes
    eblhs_in = nc.dram_tensor("eb_lhs", [128, 10, 64], BF16,
                              kind="ExternalInput")
    # gather indices, bf16 values, [3 tabs, 8 g, 4096 (jj jt i)]
    idx_in = nc.dram_tensor("idx3", [3, 8, 4096], BF16, kind="ExternalInput")

    w_names = ["Wq", "Wk", "Wv", "Wo", "W1", "W2"]
    w_ins = {n: nc.dram_tensor(n, [128, L, 2, H], BF16, kind="ExternalInput")
             for n in w_names}
    b_names = ["bq", "ln1_s", "ln1_b", "ln2_s", "ln2_b", "bv"]
    b_ins = {n: nc.dram_tensor(n, [128, L, 2], F32, kind="ExternalInput")
             for n in b_names}
    # row-broadcast biases (per free dim): bo, b1, b2
    brow_in = nc.dram_tensor("brow", [L, 3, H], F32, kind="ExternalInput")

    out_t = nc.dram_tensor("out", [T, H], F32, kind="ExternalOutput")

    CCW = H * T                   # 16384 bf16 = 32KB
    cc_ins = [nc.dram_tensor(f"cc_in{i}", [CCW], BF16) for i in range(L)]
    cc_outs = [nc.dram_tensor(f"cc_out{i}", [NC, CCW], BF16,
                              addr_space="Shared") for i in range(L)]
    ccd_in = nc.dram_tensor("ccd_in", [16], BF16)
    ccd_out = nc.dram_tensor("ccd_out", [NC, 16], BF16, addr_space="Shared")

    with tile.TileContext(nc) as tc:
        ctx = contextlib.ExitStack()
        with ctx:
            const = ctx.enter_context(tc.tile_pool(name="const", bufs=1))
            wpool = ctx.enter_context(tc.tile_pool(name="weights", bufs=1))
            small = ctx.enter_context(tc.tile_pool(name="small", bufs=2))
            psT = ctx.enter_context(tc.tile_pool(name="psT", bufs=2, space="PSUM"))
            psTP = ctx.enter_context(tc.tile_pool(name="psTP", bufs=2, space="PSUM"))

            # ---- dummy collective: absorbs entry barrier + cold start ----
            dummy = const.tile([1, 16], BF16)
            nc.vector.memset(dummy, 0.0)
            nc.sync.dma_start(out=ccd_in.ap().rearrange("(p c) -> p c", p=1),
                              in_=dummy)
            nc.gpsimd.collective_compute(
                "AllGather", AL.bypass, replica_groups=[list(range(NC))],
                ins=[ccd_in[:]], outs=[ccd_out[:, :]])

            # ---- critical-path loads first ----
            ident = const.tile([128, 128], BF16)
            nc.sync.dma_start(out=ident, in_=ident_in[:, :])
            xT = const.tile([128, 2, T], F32)
            nc.sync.dma_start(out=xT, in_=xT_in.ap().rearrange("a p t -> p a t"))
            wfeat = const.tile([128, 2, H], F32)
            nc.sync.dma_start(out=wfeat,
                              in_=wfeat_in.ap().rearrange("a p f -> p a f"))
            iota16f = const.tile([128, 1], F32)
            nc.scalar.dma_start(out=iota16f,
                                in_=iota16_in.ap().rearrange("(p o) -> p o", o=1))
            bfeat_r = const.tile([64, H], F32)
            nc.scalar.dma_start(out=bfeat_r, in_=_bcast_row(bfeat_in.ap(), T))
            bsb = {}
            for n in ["ln1_s", "ln1_b"]:
                tl = wpool.tile([128, L, 2], F32, tag="b_" + n, name="b_" + n)
                nc.scalar.dma_start(out=tl, in_=b_ins[n][:, :, :])
                bsb[n] = tl
            magic = const.tile([128, 1], I32)
            nc.vector.memset(magic, 0x5F3759DF)

            # ---------------- helpers ----------------
            def rsqrt_col(u_ap, p, tagp, iters=1):
                ki = small.tile([128, 1], I32, tag=tagp + "ki")
                nc.vector.tensor_scalar(out=ki[:p], in0=u_ap.bitcast(I32),
                                        scalar1=1, scalar2=None,
                                        op0=AL.logical_shift_right)
                z = small.tile([128, 1], F32, tag=tagp + "z")
                nc.vector.tensor_tensor(out=z[:p].bitcast(I32), in0=magic[:p],
                                        in1=ki[:p], op=AL.subtract)
                t = small.tile([128, 1], F32, tag=tagp + "t")
                for _ in range(iters):
                    nc.vector.tensor_scalar(out=t[:p], in0=z[:p], scalar1=z[:p],
                                            scalar2=u_ap, op0=AL.mult, op1=AL.mult)
                    nc.vector.tensor_scalar(out=t[:p], in0=t[:p], scalar1=-0.5,
                                            scalar2=1.5, op0=AL.mult, op1=AL.add)
                    nc.vector.tensor_tensor(out=z[:p], in0=z[:p], in1=t[:p],
                                            op=AL.mult)
                return z

            def layernorm_stats(h_ap, tagp):
                stats = small.tile([128, 6], F32, tag=tagp + "st")
                nc.vector.bn_stats(out=stats[:T], in_=h_ap)
                mv = small.tile([128, 2], F32, tag=tagp + "mv")
                nc.vector.bn_aggr(out=mv[:T], in_=stats[:T])
                u = small.tile([128, 1], F32, tag=tagp + "u")
                nc.vector.tensor_scalar(out=u[:T], in0=mv[:T, 1:2],
                                        scalar1=LN_EPS, scalar2=None, op0=AL.add)
                rstd = rsqrt_col(u[:T], T, tagp)
                return mv, rstd

            # =====================================================
            # Stage A: h0 = x @ Wfeat + b (f32); layer-0 LN1 -> yT -> send
            # =====================================================
            h_sb = const.tile([64, H], F32, tag="resid")
            h_ps = psT.tile([64, H], F32, tag="mm")
            for a in range(2):
                nc.tensor.matmul(h_ps, xT[:, a], wfeat[:, a],
                                 start=(a == 0), stop=(a == 1))
            nc.vector.tensor_tensor(out=h_sb, in0=h_ps, in1=bfeat_r,
                                    op=AL.add)

            yT_sb = const.tile([128, 2, T], BF16, tag="yT")
            y0 = const.tile([64, H], BF16, tag="y0")

            def ln1_to_yT(l):
                mv, rstd = layernorm_stats(h_sb, "ln1")
                nc.vector.tensor_scalar(out=y0, in0=h_sb,
                                        scalar1=mv[:T, 0:1], scalar2=rstd[:T],
                                        op0=AL.subtract, op1=AL.mult)
                for a in range(2):
                    tp = psTP.tile([128, T], BF16, tag="tp")
                    nc.tensor.transpose(tp, y0[:, 128 * a:128 * (a + 1)],
                                        ident[:T, :T])
                    nc.scalar.activation(yT_sb[:, a], tp, AF.Identity,
                                         bias=bsb["ln1_b"][:, l, a:a + 1],
                                         scale=bsb["ln1_s"][:, l, a:a + 1])
                nc.sync.dma_start(
                    out=cc_ins[l].ap().rearrange("(p c) -> p c", p=128),
                    in_=yT_sb.rearrange("p a t -> p (a t)"))
                nc.gpsimd.collective_compute(
                    "AllGather", AL.bypass,
                    replica_groups=[list(range(NC))],
                    ins=[cc_ins[l][:]], outs=[cc_outs[l][:, :]])

            ln1_to_yT(0)

            # =====================================================
            # Stage B: bulk loads + bias gather (overlaps barrier + AG0)
            # =====================================================
            bctx = contextlib.ExitStack()
            bb = bctx.enter_context(tc.tile_pool(name="biasbuild", bufs=1))
            ohp = bctx.enter_context(tc.tile_pool(name="ohp", bufs=2))
            psG = bctx.enter_context(tc.tile_pool(name="psG", bufs=2, space="PSUM"))

            eblhs = const.tile([128, 10, 64], BF16)
            nc.sync.dma_start(out=eblhs, in_=eblhs_in[:, :, :])
            idx_t = {}
            for k, tab in enumerate(["t1a", "t1b", "t2"]):
                it = bb.tile([128, 4096], BF16, tag="idx_" + tab,
                             name="idx_" + tab)
                eng = nc.sync if tab != "t1b" else nc.scalar
                eng.dma_start(out=it, in_=_gbcast(idx_in.ap()[k], 16, 4096))
                idx_t[tab] = it
            # remaining weights/biases, balanced across the two rings
            wsb = {}
            for i, n in enumerate(["Wq", "Wk", "Wv", "Wo", "W1", "W2"]):
                tl = wpool.tile([128, L, 2, H], BF16, tag="w_" + n,
                                name="w_" + n)
                eng = nc.scalar if i % 2 == 0 else nc.sync
                eng.dma_start(out=tl, in_=w_ins[n][:, :, :, :])
                wsb[n] = tl
            for n in ["bq", "ln2_s", "ln2_b", "bv"]:
                tl = wpool.tile([128, L, 2], F32, tag="b_" + n, name="b_" + n)
                nc.scalar.dma_start(out=tl, in_=b_ins[n][:, :, :])
                bsb[n] = tl
            brow = wpool.tile([64, L, 3, H], F32, tag="brow")
            nc.sync.dma_start(out=brow, in_=_bcast_row(
                brow_in.ap().rearrange("l k f -> (l k f)"), T).rearrange(
                    "p (l k f) -> p l k f", l=L, k=3))

            # gather: all 12 one-hot passes accumulate into one PSUM tile,
            # a single eviction yields the summed raw bias [64=(8h+g), 4096]
            bias_hm = bb.tile([64, 4096], BF16, tag="bias_hm")
            passes = [("t1a", 0, 0), ("t1a", 0, 1),
                      ("t1b", 0, 0), ("t1b", 0, 1)] + \
                     [("t2", 2, q) for q in range(8)]
            for ch in range(4):              # 1024-col chunks
                ps = psG.tile([64, 1024], F32, tag="g")
                for pi, (tab, lhs0, q) in enumerate(passes):
                    oh = ohp.tile([128, 1024], BF16, tag="oh")
                    nc.vector.tensor_scalar(
                        out=oh, in0=idx_t[tab][:, 1024 * ch:1024 * (ch + 1)],
                        scalar1=float(16 * q), scalar2=iota16f,
                        op0=AL.subtract, op1=AL.is_equal)
                    for hf in range(2):
                        nc.tensor.matmul(
                            ps[:, 512 * hf:512 * (hf + 1)],
                            eblhs[:, lhs0 + q],
                            oh[:, 512 * hf:512 * (hf + 1)],
                            start=(pi == 0), stop=(pi == len(passes) - 1))
                nc.scalar.activation(bias_hm[:, 1024 * ch:1024 * (ch + 1)],
                                     ps, AF.Copy)

            # reorient to eb [128 j, (m, jt, h', i)] (raw bias, bf16)
            eb = const.tile([128, 2, NJT, 4, T], BF16, tag="eb")
            for h in range(NH):
                m, hp = h // 4, h % 4
                eng = nc.sync if h % 2 == 0 else nc.scalar
                eng.dma_start(
                    out=eb[:, m, :, hp, :],
                    in_=bias_hm[8 * h:8 * h + 8].rearrange(
                        "g (jj r) -> g jj r", jj=16))

            bctx.close()

            # =====================================================
            # Stage C: layers
            # =====================================================
            lctx = contextlib.ExitStack()
            work = lctx.enter_context(tc.tile_pool(name="work", bufs=2))
            psS = lctx.enter_context(tc.tile_pool(name="psS", bufs=2, space="PSUM"))

            # persistent attention tiles
            qbd = const.tile([128, 2, 4 * T], BF16, tag="qbd")
            nc.vector.memset(qbd.rearrange("p a c -> p (a c)"), 0.0)
            vt = const.tile([128, NJT, NH, 33], BF16, tag="vt")
            nc.vector.memset(vt.rearrange("p j h d -> p (j h d)"), 1.0)

            for l in range(L):
                if l > 0:
                    ln1_to_yT(l)

                # ---- q block-diag (overlaps AG) ----
                q_ps = psT.tile([128, 2, T], F32, tag="mm")
                for m2 in range(2):
                    for a in range(2):
                        nc.tensor.matmul(
                            q_ps[:, m2],
                            wsb["Wq"][:, l, a, 128 * m2:128 * (m2 + 1)],
                            yT_sb[:, a], start=(a == 0), stop=(a == 1))
                for h in range(NH):
                    m2, hp = h // 4, h % 4
                    dst = qbd[32 * hp:32 * hp + 32, m2, T * hp:T * hp + T]
                    sp = q_ps[32 * hp:32 * hp + 32, m2]
                    bq_ap = bsb["bq"][32 * hp:32 * hp + 32, l, m2:m2 + 1]
                    if h % 2 == 0:
                        nc.vector.tensor_scalar(out=dst, in0=sp, scalar1=bq_ap,
                                                scalar2=None, op0=AL.add)
                    else:
                        nc.scalar.activation(dst, sp, AF.Identity, bias=bq_ap,
                                             scale=1.0)
                # constant residual biases: independent of attention output,
                # applied here while DVE idles in the collective window
                nc.vector.tensor_tensor(out=h_sb, in0=h_sb,
                                        in1=brow[:, l, 0], op=AL.add)
                nc.vector.tensor_tensor(out=h_sb, in0=h_sb,
                                        in1=brow[:, l, 2], op=AL.add)

                # ---- AG lands: read back gathered yT ----
                ygT = work.tile([128, 2, N], BF16, tag="ygT")
                for a in range(2):
                    eng = nc.sync if a == 0 else nc.scalar
                    eng.dma_start(
                        out=ygT[:, a].rearrange("p (c t) -> p c t", c=NC),
                        in_=cc_outs[l].ap().rearrange(
                            "c (p a t) -> p a c t", p=128, a=2)[:, a])

                # ---- full K^T ----
                kT = work.tile([128, 2, N], BF16, tag="kT")
                for m2 in range(2):
                    k_ps = psT.tile([128, N], F32, tag="mm")
                    for a in range(2):
                        nc.tensor.matmul(
                            k_ps, wsb["Wk"][:, l, a, 128 * m2:128 * (m2 + 1)],
                            ygT[:, a], start=(a == 0), stop=(a == 1))
                    # k-bias dropped: constant-per-row in scores, softmax-inv.
                    if m2 == 0:
                        nc.vector.tensor_copy(out=kT[:, m2], in_=k_ps)
                    else:
                        nc.scalar.activation(kT[:, m2], k_ps, AF.Copy)

                # ---- full V (token-major per j-tile), ones col persists ----
                for jt in range(NJT):
                    v_ps = psT.tile([128, H], F32, tag="mm")
                    for a in range(2):
                        nc.tensor.matmul(
                            v_ps, ygT[:, a, 128 * jt:128 * (jt + 1)],
                            wsb["Wv"][:, l, a], start=(a == 0), stop=(a == 1))
                    if jt % 2 == 0:
                        nc.vector.tensor_copy(
                            out=vt[:, jt, :, 0:32],
                            in_=v_ps.rearrange("p (h d) -> p h d", h=NH))
                    else:
                        nc.scalar.activation(
                            vt[:, jt, :, 0:32],
                            v_ps.rearrange("p (h d) -> p h d", h=NH), AF.Copy)

                # ---- scores + exp + bias-mult ----
                probs = work.tile([128, 2, NJT, 4, T], BF16, tag="probs")
                for m2 in range(2):
                    s_ps = psS.tile([128, NJT, 4, T], F32, tag="sc")
                    for jt in range(NJT):
                        nc.tensor.matmul(
                            s_ps[:, jt],
                            kT[:, m2, 128 * jt:128 * (jt + 1)],
                            qbd[:, m2], start=True, stop=False)
                    for jt in range(NJT):
                        # += bias via identity lhsT (I^T @ eb = eb)
                        nc.tensor.matmul(
                            s_ps[:, jt], ident,
                            eb[:, m2, jt].rearrange("p h t -> p (h t)"),
                            start=False, stop=True)
                    nc.scalar.activation(
                        probs[:, m2].rearrange("p j h t -> p (j h t)"),
                        s_ps.rearrange("p j h t -> p (j h t)"), AF.Exp)

                # ---- A@V with ones-col row sums ----
                o_ps = psT.tile([64, NH, 33], F32, tag="mm")
                for h in range(NH):
                    m2, hp = h // 4, h % 4
                    for jt in range(NJT):
                        nc.tensor.matmul(o_ps[:, h], probs[:, m2, jt, hp],
                                         vt[:, jt, h],
                                         start=(jt == 0), stop=(jt == NJT - 1))
                rec = small.tile([64, NH], F32, tag="rec")
                nc.vector.reciprocal(out=rec, in_=o_ps[:, :, 32])
                o_sb = work.tile([64, H], BF16, tag="o_sb")
                for h in range(NH):
                    if h % 2 == 0:
                        nc.vector.tensor_scalar(
                            out=o_sb[:, 32 * h:32 * (h + 1)],
                            in0=o_ps[:, h, 0:32],
                            scalar1=rec[:, h:h + 1], scalar2=None, op0=AL.mult)
                    else:
                        nc.scalar.activation(
                            o_sb[:, 32 * h:32 * (h + 1)], o_ps[:, h, 0:32],
                            AF.Identity, scale=rec[:, h:h + 1])

                # ---- h += (o + bv-fold) @ Wo + bo ----
                oT = work.tile([128, 2, T], BF16, tag="oT")
                for a in range(2):
                    tp = psTP.tile([128, T], BF16, tag="tp")
                    nc.tensor.transpose(tp, o_sb[:, 128 * a:128 * (a + 1)],
                                        ident[:T, :T])
                    nc.scalar.activation(oT[:, a], tp, AF.Identity,
                                         bias=bsb["bv"][:, l, a:a + 1],
                                         scale=1.0)
                at_ps = psT.tile([64, H], F32, tag="mm")
                for a in range(2):
                    nc.tensor.matmul(at_ps, oT[:, a], wsb["Wo"][:, l, a],
                                     start=(a == 0), stop=(a == 1))
                nc.vector.tensor_tensor(out=h_sb, in0=h_sb, in1=at_ps,
                                        op=AL.add)

                # ---- LN2 + FFN ----
                mv2, rstd2 = layernorm_stats(h_sb, "ln2")
                y2 = work.tile([64, H], BF16, tag="y2")
                nc.vector.tensor_scalar(out=y2, in0=h_sb,
                                        scalar1=mv2[:T, 0:1], scalar2=rstd2[:T],
                                        op0=AL.subtract, op1=AL.mult)
                y2T = work.tile([128, 2, T], BF16, tag="y2T")
                for a in range(2):
                    tp = psTP.tile([128, T], BF16, tag="tp")
                    nc.tensor.transpose(tp, y2[:, 128 * a:128 * (a + 1)],
                                        ident[:T, :T])
                    nc.scalar.activation(y2T[:, a], tp, AF.Identity,
                                         bias=bsb["ln2_b"][:, l, a:a + 1],
                                         scale=bsb["ln2_s"][:, l, a:a + 1])
                z_ps = psT.tile([64, H], F32, tag="mm")
                for a in range(2):
                    nc.tensor.matmul(z_ps, y2T[:, a], wsb["W1"][:, l, a],
                                     start=(a == 0), stop=(a == 1))
                z_sb = work.tile([64, H], BF16, tag="z")
                nc.vector.tensor_tensor(out=z_sb, in0=z_ps, in1=brow[:, l, 1],
                                        op=AL.add)
                # tanh-gelu: gg = z * (0.5 + 0.5*tanh(z*(GC1 + GC2 z^2)))
                z2 = work.tile([64, H], BF16, tag="z2")
                nc.scalar.activation(z2, z_sb, AF.Square, scale=SQ_GC2)
                zg = work.tile([64, H], BF16, tag="zg")
                nc.vector.tensor_scalar(out=zg, in0=z2, scalar1=GC1,
                                        scalar2=None, op0=AL.add)
                gu = work.tile([64, H], BF16, tag="gu")
                nc.vector.tensor_tensor(out=gu, in0=zg, in1=z_sb, op=AL.mult)
                gt = work.tile([64, H], BF16, tag="gt")
                nc.scalar.activation(gt, gu, AF.Tanh)
                gh = work.tile([64, H], BF16, tag="gh")
                nc.scalar.activation(gh, gt, AF.Copy, bias=0.5, scale=0.5)
                gg = work.tile([64, H], BF16, tag="gg")
                nc.vector.tensor_tensor(out=gg, in0=gh, in1=z_sb, op=AL.mult)
                gT = work.tile([128, 2, T], BF16, tag="gT")
                for a in range(2):
                    tp = psTP.tile([128, T], BF16, tag="tp")
                    nc.tensor.transpose(tp, gg[:, 128 * a:128 * (a + 1)],
                                        ident[:T, :T])
                    nc.vector.tensor_copy(out=gT[:, a], in_=tp)
                f_ps = psT.tile([64, H], F32, tag="mm")
                for a in range(2):
                    nc.tensor.matmul(f_ps, gT[:, a], wsb["W2"][:, l, a],
                                     start=(a == 0), stop=(a == 1))
                nc.vector.tensor_tensor(out=h_sb, in0=h_sb, in1=f_ps,
                                        op=AL.add)

            nc.sync.dma_start(out=out_t[:, :], in_=h_sb)
            lctx.close()

    nc.compile()
    return nc


# ---------------- host marshalling ----------------

def _prep_inputs(inputs):
    import ml_dtypes

    BF = ml_dtypes.bfloat16

    def f32(a):
        return np.ascontiguousarray(np.asarray(a, np.float32))

    def bf16(a):
        return np.ascontiguousarray(np.asarray(a).astype(BF))

    x = f32(inputs["x"])
    ee = np.asarray(inputs["edge_encodes"]).astype(np.int64)
    ede = np.asarray(inputs["edge_dist_encodes"]).astype(np.int64)[:, :, 0]

    # --- weight preprocessing: renorm + project + exp the bias tables ---
    def renorm(t):
        t = np.asarray(t, np.float64)
        n = np.linalg.norm(t, axis=-1, keepdims=True)
        return t * np.where(n > 1.0, 1.0 / (n + 1e-7), 1.0)

    p1 = renorm(inputs["edge_emb"]) @ np.asarray(inputs["W_ee"], np.float64)
    p2 = renorm(inputs["edge_dist_emb"]) @ np.asarray(inputs["W_ed"], np.float64)
    t1v = 0.5 * p1                                             # [32, 8]
    t2v = p2 + np.asarray(inputs["b_ee"], np.float64) \
        + np.asarray(inputs["b_ed"], np.float64)               # [128, 8]

    # block-diag lhsT tiles [10, 128, 64]: [q][16g+e', 8h+g]
    eb_lhs = np.zeros((10, 8, 16, 8, 8), np.float64)
    for q in range(2):
        for g in range(8):
            eb_lhs[q, g, :, :, g] = t1v[16 * q:16 * q + 16]
    for q in range(8):
        for g in range(8):
            eb_lhs[2 + q, g, :, :, g] = t2v[16 * q:16 * q + 16]
    # -> [128, 10, 64] partition-major for a contiguous load
    eb_lhs = np.ascontiguousarray(
        eb_lhs.reshape(10, 128, 64).transpose(1, 0, 2))

    def wprep(w):
        # [L, H, H] -> [128 p, L, 2 a, H] partition-major contiguous
        return np.ascontiguousarray(
            np.asarray(w).reshape(L, 2, 128, H).transpose(2, 0, 1, 3))

    def bprep(b):
        # [L, H] -> [128 p, L, 2 a]
        return np.ascontiguousarray(
            np.asarray(b, np.float32).reshape(L, 2, 128).transpose(2, 0, 1))

    shared = {
        "w_feat": f32(inputs["W_feat"]).reshape(2, 128, H),
        "b_feat": f32(inputs["b_feat"]),
        "identbf": np.eye(128, dtype=np.float32).astype(BF),
        "iota16": (np.arange(128) % 16).astype(np.float32),
        "eb_lhs": eb_lhs.astype(BF),
        "bq": bprep(np.asarray(inputs["bq"], np.float64) * SCALE),
        "bv": bprep(inputs["bv"]),
        "ln1_s": bprep(inputs["ln1_s"]), "ln1_b": bprep(inputs["ln1_b"]),
        "ln2_s": bprep(inputs["ln2_s"]), "ln2_b": bprep(inputs["ln2_b"]),
        "Wq": wprep(np.asarray(inputs["Wq"], np.float64) * SCALE).astype(BF),
    }
    for n in ["Wk", "Wv", "Wo", "W1", "W2"]:
        shared[n] = bf16(wprep(inputs[n]))
    shared["brow"] = np.ascontiguousarray(np.stack(
        [f32(inputs["bo"]), f32(inputs["b1"]), f32(inputs["b2"])],
        axis=1))

    in_maps = []
    for c in range(NC):
        rows = slice(T * c, T * (c + 1))
        m = dict(shared)
        m["xT"] = np.ascontiguousarray(x[rows].T).reshape(2, 128, T)
        # idx layout [tab, 8 g, (16 jj, 4 jt, 64 i)], j = jt*128 + g*16 + jj
        idx3 = np.empty((3, 8, 16, NJT, T), np.float32)
        for k, arr in enumerate([ee[rows, :, 0], ee[rows, :, 1], ede[rows]]):
            # arr [64 i, 512 j] -> [jt, g, jj, i] -> [g, jj, jt, i]
            a4 = arr.T.reshape(NJT, 8, 16, T).transpose(1, 2, 0, 3)
            idx3[k] = a4
        m["idx3"] = np.ascontiguousarray(idx3.reshape(3, 8, 4096)).astype(BF)
        in_maps.append(m)
    return in_maps


def kernel(**inputs):
    debug = inputs.pop("_debug", False)
    trace = inputs.pop("_trace", False)
    tmpdir = inputs.pop("_tmpdir", None)
    key = ("k", debug)
    if key not in _CACHE:
        _CACHE[key] = build(debug=debug)
    nc = _CACHE[key]
    in_maps = _prep_inputs(inputs)
    res = run_bass_kernel_spmd(nc, in_maps, list(range(NC)), trace=trace,
                               tmpdir=tmpdir)
    kernel._last = res
    out = np.concatenate([res.results[c]["out"] for c in range(NC)], axis=0)
    return out
